# revision 33
# baseline (speedup 1.0000x reference)
"""nn_GateMulti — MoE routing (8 experts, one-hot gate) on 8 TRN2 NeuronCores.

Strategy: expert-parallel. The gate is exactly one-hot on groups[:, 0], so
each token needs exactly one expert's MLP. Host-side "all-to-all": sort the
4096 tokens by expert id, pad each expert's token set to a common capacity,
and hand core e exactly expert e's tokens (transposed) plus expert e's
weights. Each core then runs a dense 2-layer MLP:

    yT = W2.T @ relu(W1.T @ xT + b1) + b2        (feature-major layout)

All matmul operands are bf16 (fast weight load, half the DMA bytes); PSUM
accumulation is fp32. The host scatters per-core outputs back to the
original token order.

Schedule notes (from perfetto analysis):
- The pre-stream head is DMA-bound at ~195 GB/s aggregate (early-kernel
  ramp; the two HWDGE rows fair-share it). Ring order: scalar carries
  w1[f0] + b1; sync carries the whole xT then the remaining w1 slices in
  consumption order; w2 + b2 are gated behind the first real matmul.
  Layer 1 is chunk-INTERLEAVED so w1 demand (~143 GB/s) stays under the
  early supply — a chunk-outer order (285 GB/s demand) stalls ~3us.
- Warm-up matmuls (no input deps, DVE memset sources) hold the PE HAM
  activity window busy during the input-DMA wait so the real stream starts
  at 2.4 GHz (HAM un-throttles ~5us after first PE activity).
- Layer-1 relu+bias runs on the DVE (fused tensor_scalar add+max from
  PSUM), layer-2 bias on the scalar ACT engine, so the two streams overlap
  at the layer boundary.
- Outputs leave as one full-width DMA per (c,k) tile, alternating between
  the two HWDGE rings (16 half-DMAs on one ring backlogged the sequencer
  and paced the tail); the final tile is split across scalar-ACT/sync-DMA
  and DVE-add/scalar-DMA so the two half-chains fully overlap. Outputs are
  bf16 (host casts back) to halve the tail drain.

Self-contained: shapes hardcoded from the problem spec.
"""

import math
from functools import lru_cache

import ml_dtypes
import numpy as np

import concourse.bacc as bacc
import concourse.mybir as mybir
import concourse.tile as tile
from concourse.bass_utils import run_bass_kernel_spmd

E = 8
B = 4096
D_IN = 512
D_FF = 2048
D_OUT = 512
GROUP_COL = 0

P = 128
D_T = D_IN // P   # 4  k-tiles for layer 1
F_T = D_FF // P   # 16 f-tiles (layer-1 out / layer-2 contraction)
O_T = D_OUT // P  # 4  o-tiles for layer 2

F32 = mybir.dt.float32
BF16 = mybir.dt.bfloat16

W_DT = A_DT = BF16
W_NP = ml_dtypes.bfloat16


N_WARM = 44  # dependency-free scratch matmuls (N=128) to lift the PE HAM
             # clock gate to 8/8 (2.4 GHz) while the pilot DMAs are in
             # flight; sized to drain right when the staircase pilot bytes
             # (xT chunk 0 + w1 f0) land (~10.5-11.4us)


def _emit(tc, nc, xT, w1, w2, b1t, b2t, yT, cap, n_chunks, chunk):
    add = mybir.AluOpType.add
    amax = mybir.AluOpType.max
    ident = mybir.ActivationFunctionType.Identity
    from concourse.bass import _add_dep_helper

    with (
        tc.tile_pool(name="consts", bufs=1) as cpool,
        tc.tile_pool(name="acts", bufs=1) as apool,
        tc.tile_pool(name="yout", bufs=4) as ypool,
        tc.tile_pool(name="psum_h", bufs=4, space="PSUM") as ph,
        tc.tile_pool(name="psum_y", bufs=3, space="PSUM") as py,
    ):
        # ---- input DMAs. Early-kernel DMA bandwidth is scarce and the
        # rings fair-share per packet, so ONLY the critical-path bytes may
        # flow before the PE starts: scalar carries w1[f0:f2] + b1, sync
        # carries xT chunk 0 then the rest of w1 in consumption order.
        w1_sb = cpool.tile([P, F_T, D_T, P], W_DT)   # [p, i, j, c]
        w2_sb = cpool.tile([P, O_T, F_T, P], W_DT)   # [p, k, i, c]
        xT_sb = apool.tile([P, n_chunks, D_T, chunk], A_DT)

        b1_sb = cpool.tile([P, F_T], F32)
        nc.scalar.dma_start(w1_sb[:, 0:1], w1.ap()[:, 0:1])
        nc.scalar.dma_start(b1_sb[:], b1t.ap())
        # sync-ring FIFO mirrors the staircase consumption order below:
        # chunk 0, f1, chunk 1, then the remaining w1 slices
        nc.sync.dma_start(xT_sb[:, 0], xT.ap()[:, 0])
        nc.sync.dma_start(w1_sb[:, 1:2], w1.ap()[:, 1:2])
        if n_chunks > 1:
            nc.sync.dma_start(xT_sb[:, 1:], xT.ap()[:, 1:])
        for lo, hi in [(2, 4), (4, 7), (7, 11), (11, 16)]:
            nc.sync.dma_start(w1_sb[:, lo:hi], w1.ap()[:, lo:hi])
        # bulk: w2, b2 — gated behind the first real matmul so they cannot
        # steal pilot bandwidth (they queue after w1 on the sync ring)
        bulk = []
        for k in range(O_T):
            bulk.append(nc.sync.dma_start(w2_sb[:, k], w2.ap()[:, k]))
        b2_sb = cpool.tile([P, O_T], F32)
        bulk.append(nc.sync.dma_start(b2_sb[:], b2t.ap()))

        hT_sb = apool.tile([P, F_T, cap], A_DT)

        first_mm = None
        # ---- layer 1: hT[f, c] = relu(sum_d W1[d, f] xT[d, c] + b1[f])
        # staircase: chunk 0 runs one i-group ahead of chunk 1, so the
        # stream starts on xT chunk 0 + w1 f0 alone (407KB critical bytes
        # instead of 684KB); chunk-interleaving still keeps w1 demand
        # (~143 GB/s) under the early DMA rate
        if n_chunks == 2:
            seq = [(0, 0)]
            for i in range(1, F_T):
                seq += [(i, 0), (i - 1, 1)]
            seq.append((F_T - 1, 1))
        else:
            seq = [(i, c) for i in range(F_T) for c in range(n_chunks)]
        for i, c in seq:
            if True:
                cs = slice(c * chunk, (c + 1) * chunk)
                hp = ph.tile([P, chunk], F32, name=f"hp_{i}_{c}", tag="hp")
                for j in range(D_T):
                    mm = nc.tensor.matmul(
                        hp[:],
                        w1_sb[:, i, j, :],
                        xT_sb[:, c, j, :],
                        start=(j == 0),
                        stop=(j == D_T - 1),
                    )
                    if first_mm is None:
                        first_mm = mm
                nc.vector.tensor_scalar(
                    hT_sb[:, i, cs], hp[:], b1_sb[:, i : i + 1], 0.0, add, amax
                )
        for dd in bulk:
            _add_dep_helper(
                dd.ins, first_mm.ins, sync=True, reason="hold bulk until pilot landed"
            )
        # ---- layer 2: yT[o, c] = sum_f W2[f, o] hT[f, c] + b2[o]
        # one full-width output DMA per tile, ring alternating; last tile
        # split in half across both rings so the final bytes leave ASAP
        n_out = n_chunks * O_T
        m = 0
        for c in range(n_chunks):
            cs = slice(c * chunk, (c + 1) * chunk)
            c0 = c * chunk
            for k in range(O_T):
                m += 1
                rows = slice(k * P, (k + 1) * P)
                if m < n_out:
                    yp = py.tile([P, chunk], F32, name=f"yp_{k}_{c}", tag="yp")
                    for i in range(F_T):
                        nc.tensor.matmul(
                            yp[:],
                            w2_sb[:, k, i, :],
                            hT_sb[:, i, cs],
                            start=(i == 0),
                            stop=(i == F_T - 1),
                        )
                    yo = ypool.tile([P, chunk], A_DT, name=f"yo_{k}_{c}", tag="yo")
                    eng = nc.sync if m % 2 else nc.scalar
                    nc.scalar.activation(
                        yo[:], yp[:], ident, bias=b2_sb[:, k : k + 1]
                    )
                    eng.dma_start(yT[rows, c0 : c0 + chunk], yo[:])
                else:
                    # final tile: accumulate into TWO half-width PSUM tiles
                    # (separate banks, LDWEIGHTS shared) so the two output
                    # ACTs run on scalar+DVE in parallel — one shared bank
                    # serializes its readers — and the halves drain on both
                    # rings concurrently
                    half = chunk // 2
                    ha = slice(c0, c0 + half)
                    hb = slice(c0 + half, c0 + chunk)
                    yp_a = py.tile([P, half], F32, name="yp_fa", tag="yp")
                    yp_b = py.tile([P, half], F32, name="yp_fb", tag="yp")
                    for i in range(F_T):
                        nc.tensor.matmul(
                            yp_a[:], w2_sb[:, k, i, :], hT_sb[:, i, ha],
                            start=(i == 0), stop=(i == F_T - 1),
                        )
                        nc.tensor.matmul(
                            yp_b[:], w2_sb[:, k, i, :], hT_sb[:, i, hb],
                            start=(i == 0), stop=(i == F_T - 1),
                        )
                    yo_a = ypool.tile([P, half], A_DT, name="yo_fa", tag="yo")
                    yo_b = ypool.tile([P, half], A_DT, name="yo_fb", tag="yo")
                    nc.scalar.activation(
                        yo_a[:], yp_a[:], ident, bias=b2_sb[:, k : k + 1]
                    )
                    nc.vector.tensor_scalar_add(
                        yo_b[:], yp_b[:], b2_sb[:, k : k + 1]
                    )
                    nc.sync.dma_start(yT[rows, ha], yo_a[:])
                    nc.scalar.dma_start(yT[rows, hb], yo_b[:])


@lru_cache(maxsize=4)
def _build_nc(cap, n_chunks, chunk):
    nc = bacc.Bacc("TRN2", target_bir_lowering=False, debug=False, num_devices=E)
    xT = nc.dram_tensor("xT", [P, n_chunks, D_T, chunk], A_DT, kind="ExternalInput")
    w1 = nc.dram_tensor("w1", [P, F_T, D_T, P], W_DT, kind="ExternalInput")
    w2 = nc.dram_tensor("w2", [P, O_T, F_T, P], W_DT, kind="ExternalInput")
    b1t = nc.dram_tensor("b1t", [P, F_T], F32, kind="ExternalInput")
    b2t = nc.dram_tensor("b2t", [P, O_T], F32, kind="ExternalInput")
    yT = nc.dram_tensor("yT", [D_OUT, cap], A_DT, kind="ExternalOutput")
    # ---- PE warm-up, emitted BEFORE the TileContext so it starts right
    # after the engine preamble barrier (~0.6us earlier than in-context):
    # scratch matmuls with no input deps hold the PE HAM activity window
    # busy during the input-DMA wait, so the real stream starts at 2.4 GHz.
    # The real matmuls simply queue behind these on the PE FIFO.
    warm_w = nc.alloc_sbuf_tensor("warm_w", [P, P], W_DT)
    warm_p = nc.alloc_psum_tensor("warm_p", [P, P], F32)
    wsem = nc.alloc_semaphore("warm_sem")
    nc.vector.memset(warm_w.ap(), 0.0).then_inc(wsem)
    nc.tensor.wait_ge(wsem, 1)
    for _ in range(N_WARM):
        nc.tensor.matmul(warm_p.ap(), warm_w.ap(), warm_w.ap())
    with tile.TileContext(nc) as tc:
        _emit(tc, nc, xT, w1, w2, b1t, b2t, yT, cap, n_chunks, chunk)
    nc.compile()
    return nc


def _plan_capacity(max_count):
    cap0 = max(int(max_count), 16)
    n_chunks = max(1, math.ceil(cap0 / 512))
    chunk = math.ceil(cap0 / (n_chunks * 2)) * 2
    return n_chunks * chunk, n_chunks, chunk


def _pack_w1(W1e):
    # w1img[p, i, j, c] = W1e[j*128 + p, i*128 + c]
    return np.ascontiguousarray(
        W1e.reshape(D_T, P, F_T, P).transpose(1, 2, 0, 3).astype(W_NP)
    )


def _pack_w2(W2e):
    # w2img[p, k, i, c] = W2e[i*128 + p, k*128 + c]
    return np.ascontiguousarray(
        W2e.reshape(F_T, P, O_T, P).transpose(1, 2, 0, 3).astype(W_NP)
    )


def _shard(x, groups, W1, b1, W2, b2):
    idx = np.asarray(groups)[:, GROUP_COL].astype(np.int64)
    order = np.argsort(idx, kind="stable")
    counts = np.bincount(idx, minlength=E)
    cap, n_chunks, chunk = _plan_capacity(counts.max())
    offs = np.concatenate([[0], np.cumsum(counts)])

    x = np.asarray(x, dtype=np.float32)
    W1 = np.asarray(W1, dtype=np.float32)
    b1 = np.asarray(b1, dtype=np.float32)
    W2 = np.asarray(W2, dtype=np.float32)
    b2 = np.asarray(b2, dtype=np.float32)

    in_maps, tok_ids = [], []
    for e in range(E):
        ids = order[offs[e] : offs[e + 1]]
        tok_ids.append(ids)
        xT = np.zeros((D_IN, cap), np.float32)
        xT[:, : len(ids)] = x[ids].T
        # pack to the SBUF image [p, c, j, x] so the single xT DMA moves
        # multi-KB contiguous lines on both sides
        xT = np.ascontiguousarray(
            xT.reshape(D_T, P, n_chunks, chunk)
            .transpose(1, 2, 0, 3)
            .astype(W_NP)
        )
        in_maps.append(
            {
                "xT": xT,
                "w1": _pack_w1(W1[e]),
                "w2": _pack_w2(W2[e]),
                "b1t": np.ascontiguousarray(b1[e].reshape(F_T, P).T),
                "b2t": np.ascontiguousarray(b2[e].reshape(O_T, P).T),
            }
        )
    return in_maps, tok_ids, counts, cap, n_chunks, chunk


def _run(x, groups, W1, b1, W2, b2, trace=False, **spmd_kwargs):
    in_maps, tok_ids, counts, cap, n_chunks, chunk = _shard(x, groups, W1, b1, W2, b2)
    nc = _build_nc(cap, n_chunks, chunk)
    res = run_bass_kernel_spmd(
        nc, in_maps, core_ids=list(range(E)), trace=trace, **spmd_kwargs
    )
    out = np.zeros((B, D_OUT), np.float32)
    for e in range(E):
        yTe = res.results[e]["yT"]
        out[tok_ids[e]] = yTe[:, : counts[e]].T.astype(np.float32)
    return out, res


def kernel(x, groups, W1, b1, W2, b2):
    out, _ = _run(x, groups, W1, b1, W2, b2)
    return out
